# revision 13
# baseline (speedup 1.0000x reference)
"""Trainium2 Bass kernel: LSTM encoder-decoder (IoT anomaly detector).

Reference semantics (B=256, T=512, I=128, H=256):
  encoder LSTM over x[B,T,I] -> final (h,c); pred_last = sigmoid(h @ lin_W.T + lin_b)
  decoder LSTM run T-1 steps feeding back its own prediction; outputs in
  forward time order [B,T,I].

Sharding: pure data parallelism, batch 256 -> 8 cores x 32.

The recurrence is latency-bound: one dependency cycle (h -> whh matmuls ->
sigmoid -> c update -> tanh -> h) per time step; batch/stream splitting
cannot raise throughput, so the kernel minimizes that cycle:
  - transposed layout: gate/hidden dims on partitions, batch (32) on free.
  - gates split across TWO psum banks: bank A = [f0 f1 i0 i1] (cols 4x32),
    bank B = [g0 g1 o0 o1].  sigmoid(f,i) waits only on bank A's 8 whh
    matmuls; bank B's 8 whh matmuls run while sigmoid(f,i) executes.
  - nonlin chain (all elementwise on DVE, in-order): fc = sig_f*c (runs
    under ACT tanh_g), u = sig_i*G, c = fc+u, ACT tanh_c, h = sig_o*Tc
    (bf16 out, feeds next step's matmuls directly).  S/G/O/Tc/u are bf16
    (2x DVE mode where both operands are 16-bit); c stays fp32 in its own
    PSUM bank (cheaper ACT tanh source, no PE writer so no collisions).
  - bias enters via ONE indicator matmul per bank (lhsT rows 0..3 hi /
    8..11 lo bf16 split), emitted with the x matmuls BEFORE the
    h-dependent whh matmuls so they execute during the previous step's
    nonlinearity.
  - decoder linear: z matmuls emitted right after h; sigmoid (bias via the
    fp32 ACT bias AP) writes the bf16 pred directly into the preds buffer,
    which is also the wih rhs.  preds DMA out as bf16; host converts.
"""

import numpy as np
import ml_dtypes

B, T, I, H = 256, 512, 128, 256
NCORES = 8
LB = B // NCORES  # 32 local batch

BF16 = ml_dtypes.bfloat16

_BUILT = {}


def _build(t_steps):
    import concourse.bass as bass
    import concourse.tile as tile
    from concourse import bacc, mybir

    f32 = mybir.dt.float32
    bf16 = mybir.dt.bfloat16
    AF = mybir.ActivationFunctionType

    nc = bacc.Bacc(
        "TRN2", target_bir_lowering=False, debug=False, num_devices=NCORES
    )

    xT_d = nc.dram_tensor("xT", [128, t_steps * LB], bf16, kind="ExternalInput")
    wih_e_d = nc.dram_tensor("wih_e", [128, 8 * 128], bf16, kind="ExternalInput")
    whh_e_d = nc.dram_tensor("whh_e", [128, 16 * 128], bf16, kind="ExternalInput")
    bfa_e_d = nc.dram_tensor("bfa_e", [128, 128], bf16, kind="ExternalInput")
    bfb_e_d = nc.dram_tensor("bfb_e", [128, 128], bf16, kind="ExternalInput")
    wih_d_d = nc.dram_tensor("wih_d", [128, 8 * 128], bf16, kind="ExternalInput")
    whh_d_d = nc.dram_tensor("whh_d", [128, 16 * 128], bf16, kind="ExternalInput")
    bfa_d_d = nc.dram_tensor("bfa_d", [128, 128], bf16, kind="ExternalInput")
    bfb_d_d = nc.dram_tensor("bfb_d", [128, 128], bf16, kind="ExternalInput")
    wlin_d = nc.dram_tensor("wlin", [128, 2 * 128], bf16, kind="ExternalInput")
    bl_d = nc.dram_tensor("bl", [128, 1], f32, kind="ExternalInput")
    ind_d = nc.dram_tensor("ind", [128, 128], bf16, kind="ExternalInput")
    out_d = nc.dram_tensor("out", [128, t_steps * LB], bf16, kind="ExternalOutput")

    with tile.TileContext(nc) as tc:
        from contextlib import ExitStack

        with ExitStack() as ctx:
            const = ctx.enter_context(tc.tile_pool(name="const", bufs=1))
            work = ctx.enter_context(tc.tile_pool(name="work", bufs=3))
            pa = ctx.enter_context(tc.tile_pool(name="pa", bufs=2, space="PSUM"))
            pb = ctx.enter_context(tc.tile_pool(name="pb", bufs=2, space="PSUM"))
            pl = ctx.enter_context(tc.tile_pool(name="pl", bufs=2, space="PSUM"))
            pc = ctx.enter_context(tc.tile_pool(name="pc", bufs=1, space="PSUM"))
            pd = ctx.enter_context(tc.tile_pool(name="pd", bufs=1, space="PSUM"))

            def load(dram, shape, dt):
                t = const.tile(shape, dt, tag=dram.name)
                nc.sync.dma_start(out=t[:], in_=dram[:])
                return t

            xT = const.tile([128, t_steps * LB], bf16, tag="xT")
            nq = 4 if t_steps % 4 == 0 else 1
            csz = (t_steps // nq) * LB

            def xchunk(q):
                nc.sync.dma_start(
                    out=xT[:, csz * q : csz * (q + 1)],
                    in_=xT_d[:, csz * q : csz * (q + 1)],
                )

            # encoder-critical tensors first so step 0 starts ASAP
            bfa_e = load(bfa_e_d, [128, 128], bf16)
            bfb_e = load(bfb_e_d, [128, 128], bf16)
            ind = load(ind_d, [128, 128], bf16)
            wih_e = load(wih_e_d, [128, 8 * 128], bf16)
            whh_e = load(whh_e_d, [128, 16 * 128], bf16)
            xchunk(0)
            for q in range(1, nq):
                xchunk(q)
            # decoder-only tensors load behind the encoder's first steps
            wih_dd = load(wih_d_d, [128, 8 * 128], bf16)
            whh_dd = load(whh_d_d, [128, 16 * 128], bf16)
            bfa_dd = load(bfa_d_d, [128, 128], bf16)
            bfb_dd = load(bfb_d_d, [128, 128], bf16)
            wlin = load(wlin_d, [128, 2 * 128], bf16)
            bl = load(bl_d, [128, 1], f32)

            preds = const.tile([128, t_steps * LB], bf16, tag="preds")
            c_full = pc.tile([128, 512], f32, tag="c", name="c_full")
            c = c_full[:, 0:64]
            h = const.tile([128, 64], bf16, tag="h")

            dmy = pd.tile([128, 512], f32, tag="dmy")
            fsc = const.tile([128, 1], f32, tag="fsc")

            nc.vector.memset(c[:], 0.0)
            nc.vector.memset(h[:], 0.0)

            cur = {"A": None, "B": None}

            def pslice(t):
                return preds[:, LB * t : LB * (t + 1)]

            def fills(bfa, bfb, x_t=None, wih=None):
                """open gates(t) groups on both banks: bias fill (+ encoder
                x matmuls). No h/pred deps -> run during previous nonlin."""
                ga = pa.tile([128, 512], f32, tag="ga")
                gb = pb.tile([128, 512], f32, tag="gb")
                cur["A"], cur["B"] = ga, gb
                nc.tensor.matmul(ga[:, 0:128], bfa[:], ind[:], start=True, stop=False)
                nc.tensor.matmul(gb[:, 0:128], bfb[:], ind[:], start=True, stop=False)
                if x_t is not None:
                    rhs = xT[:, LB * x_t : LB * (x_t + 1)]
                    for m in range(8):
                        g = cur["A" if m < 4 else "B"]
                        nc.tensor.matmul(
                            g[:, 32 * (m % 4) : 32 * (m % 4) + 32],
                            wih[:, 128 * m : 128 * (m + 1)],
                            rhs,
                            start=False,
                            stop=False,
                        )

            def whh_mms(whh, bank, stop):
                g = cur[bank]
                ms = range(0, 4) if bank == "A" else range(4, 8)
                for k in range(2):
                    for j, m in enumerate(ms):
                        last = stop and k == 1 and j == 3
                        nc.tensor.matmul(
                            g[:, 32 * (m % 4) : 32 * (m % 4) + 32],
                            whh[:, 128 * (8 * k + m) : 128 * (8 * k + m + 1)],
                            h[:, 32 * k : 32 * k + 32],
                            start=False,
                            stop=last,
                        )

            def wih_mms(wih, in_slot, bank):
                g = cur[bank]
                rhs = pslice(in_slot)
                ms = range(0, 4) if bank == "A" else range(4, 8)
                for j, m in enumerate(ms):
                    nc.tensor.matmul(
                        g[:, 32 * (m % 4) : 32 * (m % 4) + 32],
                        wih[:, 128 * m : 128 * (m + 1)],
                        rhs,
                        start=False,
                        stop=(j == 3),
                    )

            def filler():
                # dummy 1-col matmul on h (runs on PE before the whh burst)
                # + 1-col ACT op: ScalarE is mid-op when bank A closes, so
                # sigma(f,i)'s dispatch overlaps the filler tail (~90ns).
                nc.tensor.matmul(
                    dmy[:, 0:1], ind[:], h[:, 0:1], start=True, stop=True
                )
                nc.scalar.activation(fsc[:], dmy[:, 0:1], AF.Sigmoid)

            def nonlin():
                ga, gb = cur["A"], cur["B"]
                S = work.tile([128, 128], bf16, tag="S")
                nc.scalar.activation(S[:], ga[:, 0:128], AF.Sigmoid)
                G = work.tile([128, 64], bf16, tag="G")
                nc.scalar.activation(G[:], gb[:, 0:64], AF.Tanh)
                O = work.tile([128, 64], bf16, tag="O")
                nc.scalar.activation(O[:], gb[:, 64:128], AF.Sigmoid)
                fc = work.tile([128, 64], f32, tag="fc")
                nc.vector.tensor_mul(fc[:], S[:, 0:64], c[:])
                u = work.tile([128, 64], bf16, tag="u")
                nc.vector.tensor_mul(u[:], S[:, 64:128], G[:])
                nc.vector.tensor_add(c[:], fc[:], u[:])
                Tc = work.tile([128, 64], bf16, tag="Tc")
                nc.scalar.activation(Tc[:], c[:], AF.Tanh)
                nc.vector.tensor_mul(h[:], O[:], Tc[:])

            def lin_block(out_slot):
                lz = pl.tile([128, 512], f32, tag="lz")
                nc.tensor.matmul(
                    lz[:, 0:LB], wlin[:, 0:128], h[:, 0:32],
                    start=True, stop=False,
                )
                nc.tensor.matmul(
                    lz[:, 0:LB], wlin[:, 128:256], h[:, 32:64],
                    start=False, stop=True,
                )
                nc.scalar.activation(
                    pslice(out_slot), lz[:, 0:LB], AF.Sigmoid, bias=bl[:]
                )

            # ---- encoder ----
            fills(bfa_e, bfb_e, x_t=0, wih=wih_e)
            for t in range(t_steps):
                whh_mms(whh_e, "A", stop=True)
                whh_mms(whh_e, "B", stop=True)
                nonlin()
                if t + 1 < t_steps:
                    fills(bfa_e, bfb_e, x_t=t + 1, wih=wih_e)

            # pred at last slot from encoder final h; open first decoder banks
            fills(bfa_dd, bfb_dd)
            lin_block(t_steps - 1)

            # ---- decoder: iteration k consumes pred slot T-1-k, writes T-2-k
            for k in range(t_steps - 1):
                whh_mms(whh_dd, "A", stop=False)
                whh_mms(whh_dd, "B", stop=False)
                wih_mms(wih_dd, t_steps - 1 - k, "A")
                wih_mms(wih_dd, t_steps - 1 - k, "B")
                nonlin()
                if k + 1 < t_steps - 1:
                    fills(bfa_dd, bfb_dd)
                lin_block(t_steps - 2 - k)
                if t_steps >= 8:
                    q4 = t_steps // 4
                    # quarter [slot_lo, slot_hi) finished once slot_lo written
                    for slot_lo in (3 * q4, 2 * q4, q4):
                        if t_steps - 2 - k == slot_lo:
                            nc.sync.dma_start(
                                out=out_d[:, slot_lo * LB : (slot_lo + q4) * LB],
                                in_=preds[:, slot_lo * LB : (slot_lo + q4) * LB],
                            )

            lastq = (t_steps // 4) * LB if t_steps >= 8 else t_steps * LB
            nc.sync.dma_start(out=out_d[:, 0:lastq], in_=preds[:, 0:lastq])

    nc.compile()
    return nc


def _get(t_steps):
    if t_steps not in _BUILT:
        _BUILT[t_steps] = _build(t_steps)
    return _BUILT[t_steps]


def _pack_weights(enc_W_ih, enc_W_hh, enc_b_ih, enc_b_hh,
                  dec_W_ih, dec_W_hh, dec_b_ih, dec_b_hh, lin_W, lin_b):
    # chunk order [f0 f1 i0 i1 g0 g1 o0 o1]; torch gate rows are [i f g o].
    perm = np.r_[H : 2 * H, 0:H, 2 * H : 3 * H, 3 * H : 4 * H]

    def pack_ih(W):  # [4H, I] -> [128, 8*128] lhsT tiles
        Wp = W[perm].reshape(8, 128, I)
        return np.concatenate([Wp[m].T for m in range(8)], axis=1).astype(BF16)

    def pack_hh(W):  # [4H, H] -> [128, 16*128], tile (k,m) at col 128*(8k+m)
        Wp = W[perm]
        tiles = [
            Wp[128 * m : 128 * (m + 1), 128 * k : 128 * (k + 1)].T
            for k in range(2)
            for m in range(8)
        ]
        return np.concatenate(tiles, axis=1).astype(BF16)

    def pack_bias_fill(b, bank):  # [4H] -> [128,128]: rows 0..3 hi, 8..11 lo
        bp = b[perm].astype(np.float32)
        out = np.zeros((128, 128), np.float32)
        base = 0 if bank == "A" else 4
        for j in range(4):
            chunk = bp[128 * (base + j) : 128 * (base + j + 1)]
            hi = chunk.astype(BF16).astype(np.float32)
            out[j, :] = hi
            out[8 + j, :] = chunk - hi
        return out.astype(BF16)

    ind = np.zeros((128, 128), np.float32)
    for j in range(4):
        ind[j, 32 * j : 32 * j + 32] = 1.0
        ind[8 + j, 32 * j : 32 * j + 32] = 1.0

    wlin = np.concatenate(
        [lin_W[:, 0:128].T, lin_W[:, 128:256].T], axis=1
    ).astype(BF16)

    b_e = enc_b_ih + enc_b_hh
    b_d = dec_b_ih + dec_b_hh
    return {
        "wih_e": pack_ih(enc_W_ih),
        "whh_e": pack_hh(enc_W_hh),
        "bfa_e": pack_bias_fill(b_e, "A"),
        "bfb_e": pack_bias_fill(b_e, "B"),
        "wih_d": pack_ih(dec_W_ih),
        "whh_d": pack_hh(dec_W_hh),
        "bfa_d": pack_bias_fill(b_d, "A"),
        "bfb_d": pack_bias_fill(b_d, "B"),
        "wlin": wlin,
        "bl": lin_b.astype(np.float32).reshape(128, 1),
        "ind": ind.astype(BF16),
    }


def _run(inputs, t_steps, trace=False):
    from concourse.bass_utils import run_bass_kernel_spmd

    nc = _get(t_steps)
    x = np.asarray(inputs["x"], np.float32)
    shared = _pack_weights(
        np.asarray(inputs["enc_W_ih"], np.float32),
        np.asarray(inputs["enc_W_hh"], np.float32),
        np.asarray(inputs["enc_b_ih"], np.float32),
        np.asarray(inputs["enc_b_hh"], np.float32),
        np.asarray(inputs["dec_W_ih"], np.float32),
        np.asarray(inputs["dec_W_hh"], np.float32),
        np.asarray(inputs["dec_b_ih"], np.float32),
        np.asarray(inputs["dec_b_hh"], np.float32),
        np.asarray(inputs["lin_W"], np.float32),
        np.asarray(inputs["lin_b"], np.float32),
    )
    in_maps = []
    for j in range(NCORES):
        xs = x[LB * j : LB * (j + 1), :t_steps]  # [32, T, 128]
        xT = np.ascontiguousarray(xs.transpose(2, 1, 0)).reshape(128, t_steps * LB)
        m = dict(shared)
        m["xT"] = xT.astype(BF16)
        in_maps.append(m)

    res = run_bass_kernel_spmd(
        nc, in_maps, list(range(NCORES)), trace=trace
    )
    out = np.empty((B, t_steps, I), np.float32)
    for j in range(NCORES):
        o = res.results[j]["out"].astype(np.float32).reshape(128, t_steps, LB)
        out[LB * j : LB * (j + 1)] = o.transpose(2, 1, 0)
    return out, res


def kernel(**inputs):
    out, _ = _run(inputs, T)
    return out


# revision 15
# speedup vs baseline: 1.1969x; 1.1969x over previous
"""Trainium2 Bass kernel: LSTM encoder-decoder (IoT anomaly detector).

Reference semantics (B=256, T=512, I=128, H=256):
  encoder LSTM over x[B,T,I] -> final (h,c); pred_last = sigmoid(h @ lin_W.T + lin_b)
  decoder LSTM run T-1 steps feeding back its own prediction; outputs in
  forward time order [B,T,I].

Sharding: pure data parallelism, batch 256 -> 8 cores x 32.

The recurrence is latency-bound: one dependency cycle (h -> whh matmuls ->
sigmoid -> c update -> tanh -> h) per time step; batch/stream splitting
cannot raise throughput, so the kernel minimizes that cycle:
  - transposed layout: gate/hidden dims on partitions, batch (32) on free.
  - gates split across TWO psum banks: bank A = [f0 f1 i0 i1] (cols 4x32),
    bank B = [g0 g1 o0 o1].  sigmoid(f,i) waits only on bank A's 8 whh
    matmuls; bank B's 8 whh matmuls run while sigmoid(f,i) executes.
  - nonlin chain (all elementwise on DVE, in-order): fc = sig_f*c (runs
    under ACT tanh_g), u = sig_i*G, c = fc+u, ACT tanh_c, h = sig_o*Tc
    (bf16 out, feeds next step's matmuls directly).  S/G/O/Tc/u are bf16
    (2x DVE mode where both operands are 16-bit); c stays fp32 in its own
    PSUM bank (cheaper ACT tanh source, no PE writer so no collisions).
  - bias enters via ONE indicator matmul per bank (lhsT rows 0..3 hi /
    8..11 lo bf16 split), emitted with the x matmuls BEFORE the
    h-dependent whh matmuls so they execute during the previous step's
    nonlinearity.
  - decoder linear: z matmuls emitted right after h; sigmoid (bias via the
    fp32 ACT bias AP) writes the bf16 pred directly into the preds buffer,
    which is also the wih rhs.  preds DMA out as bf16; host converts.
"""

import numpy as np
import ml_dtypes

B, T, I, H = 256, 512, 128, 256
NCORES = 8
LB = B // NCORES  # 32 local batch

BF16 = ml_dtypes.bfloat16

_BUILT = {}


def _build(t_steps):
    import concourse.bass as bass
    import concourse.tile as tile
    from concourse import bacc, mybir

    f32 = mybir.dt.float32
    bf16 = mybir.dt.bfloat16
    AF = mybir.ActivationFunctionType

    nc = bacc.Bacc(
        "TRN2", target_bir_lowering=False, debug=False, num_devices=NCORES
    )

    xT_d = nc.dram_tensor("xT", [128, t_steps * LB], bf16, kind="ExternalInput")
    wih_e_d = nc.dram_tensor("wih_e", [128, 8 * 128], bf16, kind="ExternalInput")
    whh_e_d = nc.dram_tensor("whh_e", [128, 16 * 128], bf16, kind="ExternalInput")
    bfa_e_d = nc.dram_tensor("bfa_e", [128, 128], bf16, kind="ExternalInput")
    bfb_e_d = nc.dram_tensor("bfb_e", [128, 128], bf16, kind="ExternalInput")
    wih_d_d = nc.dram_tensor("wih_d", [128, 8 * 128], bf16, kind="ExternalInput")
    whh_d_d = nc.dram_tensor("whh_d", [128, 16 * 128], bf16, kind="ExternalInput")
    bfa_d_d = nc.dram_tensor("bfa_d", [128, 128], bf16, kind="ExternalInput")
    bfb_d_d = nc.dram_tensor("bfb_d", [128, 128], bf16, kind="ExternalInput")
    wlin_d = nc.dram_tensor("wlin", [128, 2 * 128], bf16, kind="ExternalInput")
    bl_d = nc.dram_tensor("bl", [128, 1], f32, kind="ExternalInput")
    ind_d = nc.dram_tensor("ind", [128, 128], bf16, kind="ExternalInput")
    out_d = nc.dram_tensor("out", [128, t_steps * LB], bf16, kind="ExternalOutput")

    with tile.TileContext(nc) as tc:
        from contextlib import ExitStack

        with ExitStack() as ctx:
            const = ctx.enter_context(tc.tile_pool(name="const", bufs=1))
            work = ctx.enter_context(tc.tile_pool(name="work", bufs=3))
            pa = ctx.enter_context(tc.tile_pool(name="pa", bufs=2, space="PSUM"))
            pb = ctx.enter_context(tc.tile_pool(name="pb", bufs=2, space="PSUM"))
            pl = ctx.enter_context(tc.tile_pool(name="pl", bufs=2, space="PSUM"))
            pc = ctx.enter_context(tc.tile_pool(name="pc", bufs=1, space="PSUM"))
            pd = ctx.enter_context(tc.tile_pool(name="pd", bufs=1, space="PSUM"))

            def load(dram, shape, dt):
                t = const.tile(shape, dt, tag=dram.name)
                nc.sync.dma_start(out=t[:], in_=dram[:])
                return t

            xT = const.tile([128, t_steps * LB], bf16, tag="xT")
            nq = 4 if t_steps % 4 == 0 else 1
            csz = (t_steps // nq) * LB

            # encoder-critical tensors first so step 0 starts ASAP
            bfa_e = load(bfa_e_d, [128, 128], bf16)
            bfb_e = load(bfb_e_d, [128, 128], bf16)
            ind = load(ind_d, [128, 128], bf16)
            wih_e = load(wih_e_d, [128, 8 * 128], bf16)
            whh_e = load(whh_e_d, [128, 16 * 128], bf16)
            for q in range(nq):
                nc.sync.dma_start(
                    out=xT[:, csz * q : csz * (q + 1)],
                    in_=xT_d[:, csz * q : csz * (q + 1)],
                )
            # decoder-only tensors load behind the encoder's first steps
            wih_dd = load(wih_d_d, [128, 8 * 128], bf16)
            whh_dd = load(whh_d_d, [128, 16 * 128], bf16)
            bfa_dd = load(bfa_d_d, [128, 128], bf16)
            bfb_dd = load(bfb_d_d, [128, 128], bf16)
            wlin = load(wlin_d, [128, 2 * 128], bf16)
            bl = load(bl_d, [128, 1], f32)

            preds = const.tile([128, t_steps * LB], bf16, tag="preds")
            c_full = pc.tile([128, 512], f32, tag="c", name="c_full")
            c = c_full[:, 0:64]
            h = const.tile([128, 64], bf16, tag="h")

            dmy = pd.tile([128, 512], f32, tag="dmy")
            fsc = const.tile([128, 1], f32, tag="fsc")

            nc.vector.memset(c[:], 0.0)
            nc.vector.memset(h[:], 0.0)

            cur = {"A": None, "B": None}

            def pslice(t):
                return preds[:, LB * t : LB * (t + 1)]

            def fills(bfa, bfb, x_t=None, wih=None):
                """open gates(t) groups on both banks: bias fill (+ encoder
                x matmuls). No h/pred deps -> run during previous nonlin."""
                ga = pa.tile([128, 512], f32, tag="ga")
                gb = pb.tile([128, 512], f32, tag="gb")
                cur["A"], cur["B"] = ga, gb
                nc.tensor.matmul(ga[:, 0:128], bfa[:], ind[:], start=True, stop=False)
                nc.tensor.matmul(gb[:, 0:128], bfb[:], ind[:], start=True, stop=False)
                if x_t is not None:
                    rhs = xT[:, LB * x_t : LB * (x_t + 1)]
                    for m in range(8):
                        g = cur["A" if m < 4 else "B"]
                        nc.tensor.matmul(
                            g[:, 32 * (m % 4) : 32 * (m % 4) + 32],
                            wih[:, 128 * m : 128 * (m + 1)],
                            rhs,
                            start=False,
                            stop=False,
                        )

            def whh_mms(whh, bank, stop):
                g = cur[bank]
                ms = range(0, 4) if bank == "A" else range(4, 8)
                for k in range(2):
                    for j, m in enumerate(ms):
                        last = stop and k == 1 and j == 3
                        nc.tensor.matmul(
                            g[:, 32 * (m % 4) : 32 * (m % 4) + 32],
                            whh[:, 128 * (8 * k + m) : 128 * (8 * k + m + 1)],
                            h[:, 32 * k : 32 * k + 32],
                            start=False,
                            stop=last,
                        )

            def wih_mms(wih, in_slot, bank):
                g = cur[bank]
                rhs = pslice(in_slot)
                ms = range(0, 4) if bank == "A" else range(4, 8)
                for j, m in enumerate(ms):
                    nc.tensor.matmul(
                        g[:, 32 * (m % 4) : 32 * (m % 4) + 32],
                        wih[:, 128 * m : 128 * (m + 1)],
                        rhs,
                        start=False,
                        stop=(j == 3),
                    )

            def filler():
                # dummy 1-col matmul on h (runs on PE before the whh burst)
                # + 1-col ACT op: ScalarE is mid-op when bank A closes, so
                # sigma(f,i)'s dispatch overlaps the filler tail (~90ns).
                nc.tensor.matmul(
                    dmy[:, 0:1], ind[:], h[:, 0:1], start=True, stop=True
                )
                nc.scalar.activation(fsc[:], dmy[:, 0:1], AF.Sigmoid)

            def nonlin():
                ga, gb = cur["A"], cur["B"]
                S = work.tile([128, 128], bf16, tag="S")
                nc.scalar.activation(S[:], ga[:, 0:128], AF.Sigmoid)
                G = work.tile([128, 64], bf16, tag="G")
                nc.scalar.activation(G[:], gb[:, 0:64], AF.Tanh)
                O = work.tile([128, 64], bf16, tag="O")
                nc.scalar.activation(O[:], gb[:, 64:128], AF.Sigmoid)
                fc = work.tile([128, 64], f32, tag="fc")
                nc.vector.tensor_mul(fc[:], S[:, 0:64], c[:])
                u = work.tile([128, 64], bf16, tag="u")
                nc.vector.tensor_mul(u[:], S[:, 64:128], G[:])
                nc.vector.tensor_add(c[:], fc[:], u[:])
                Tc = work.tile([128, 64], bf16, tag="Tc")
                nc.scalar.activation(Tc[:], c[:], AF.Tanh)
                nc.vector.tensor_mul(h[:], O[:], Tc[:])

            def lin_block(out_slot):
                lz = pl.tile([128, 512], f32, tag="lz")
                nc.tensor.matmul(
                    lz[:, 0:LB], wlin[:, 0:128], h[:, 0:32],
                    start=True, stop=False,
                )
                nc.tensor.matmul(
                    lz[:, 0:LB], wlin[:, 128:256], h[:, 32:64],
                    start=False, stop=True,
                )
                nc.scalar.activation(
                    pslice(out_slot), lz[:, 0:LB], AF.Sigmoid, bias=bl[:]
                )

            # ---- encoder ----
            fills(bfa_e, bfb_e, x_t=0, wih=wih_e)
            for t in range(t_steps):
                whh_mms(whh_e, "A", stop=True)
                whh_mms(whh_e, "B", stop=True)
                nonlin()
                if t + 1 < t_steps:
                    fills(bfa_e, bfb_e, x_t=t + 1, wih=wih_e)

            # pred at last slot from encoder final h; open first decoder banks
            fills(bfa_dd, bfb_dd)
            lin_block(t_steps - 1)

            # ---- decoder: iteration k consumes pred slot T-1-k, writes T-2-k
            for k in range(t_steps - 1):
                whh_mms(whh_dd, "A", stop=False)
                whh_mms(whh_dd, "B", stop=False)
                wih_mms(wih_dd, t_steps - 1 - k, "A")
                wih_mms(wih_dd, t_steps - 1 - k, "B")
                nonlin()
                if k + 1 < t_steps - 1:
                    fills(bfa_dd, bfb_dd)
                lin_block(t_steps - 2 - k)
                if t_steps >= 8 and k == t_steps - 2 - (t_steps // 2):
                    half = (t_steps // 2) * LB
                    nc.sync.dma_start(
                        out=out_d[:, half:], in_=preds[:, half:]
                    )

            half = (t_steps // 2) * LB if t_steps >= 8 else 0
            nc.sync.dma_start(out=out_d[:, 0:half] if half else out_d[:],
                              in_=preds[:, 0:half] if half else preds[:])

    nc.compile()
    return nc


def _get(t_steps):
    if t_steps not in _BUILT:
        _BUILT[t_steps] = _build(t_steps)
    return _BUILT[t_steps]


def _pack_weights(enc_W_ih, enc_W_hh, enc_b_ih, enc_b_hh,
                  dec_W_ih, dec_W_hh, dec_b_ih, dec_b_hh, lin_W, lin_b):
    # chunk order [f0 f1 i0 i1 g0 g1 o0 o1]; torch gate rows are [i f g o].
    perm = np.r_[H : 2 * H, 0:H, 2 * H : 3 * H, 3 * H : 4 * H]

    def pack_ih(W):  # [4H, I] -> [128, 8*128] lhsT tiles
        Wp = W[perm].reshape(8, 128, I)
        return np.concatenate([Wp[m].T for m in range(8)], axis=1).astype(BF16)

    def pack_hh(W):  # [4H, H] -> [128, 16*128], tile (k,m) at col 128*(8k+m)
        Wp = W[perm]
        tiles = [
            Wp[128 * m : 128 * (m + 1), 128 * k : 128 * (k + 1)].T
            for k in range(2)
            for m in range(8)
        ]
        return np.concatenate(tiles, axis=1).astype(BF16)

    def pack_bias_fill(b, bank):  # [4H] -> [128,128]: rows 0..3 hi, 8..11 lo
        bp = b[perm].astype(np.float32)
        out = np.zeros((128, 128), np.float32)
        base = 0 if bank == "A" else 4
        for j in range(4):
            chunk = bp[128 * (base + j) : 128 * (base + j + 1)]
            hi = chunk.astype(BF16).astype(np.float32)
            out[j, :] = hi
            out[8 + j, :] = chunk - hi
        return out.astype(BF16)

    ind = np.zeros((128, 128), np.float32)
    for j in range(4):
        ind[j, 32 * j : 32 * j + 32] = 1.0
        ind[8 + j, 32 * j : 32 * j + 32] = 1.0

    wlin = np.concatenate(
        [lin_W[:, 0:128].T, lin_W[:, 128:256].T], axis=1
    ).astype(BF16)

    b_e = enc_b_ih + enc_b_hh
    b_d = dec_b_ih + dec_b_hh
    return {
        "wih_e": pack_ih(enc_W_ih),
        "whh_e": pack_hh(enc_W_hh),
        "bfa_e": pack_bias_fill(b_e, "A"),
        "bfb_e": pack_bias_fill(b_e, "B"),
        "wih_d": pack_ih(dec_W_ih),
        "whh_d": pack_hh(dec_W_hh),
        "bfa_d": pack_bias_fill(b_d, "A"),
        "bfb_d": pack_bias_fill(b_d, "B"),
        "wlin": wlin,
        "bl": lin_b.astype(np.float32).reshape(128, 1),
        "ind": ind.astype(BF16),
    }


def _run(inputs, t_steps, trace=False):
    from concourse.bass_utils import run_bass_kernel_spmd

    nc = _get(t_steps)
    x = np.asarray(inputs["x"], np.float32)
    shared = _pack_weights(
        np.asarray(inputs["enc_W_ih"], np.float32),
        np.asarray(inputs["enc_W_hh"], np.float32),
        np.asarray(inputs["enc_b_ih"], np.float32),
        np.asarray(inputs["enc_b_hh"], np.float32),
        np.asarray(inputs["dec_W_ih"], np.float32),
        np.asarray(inputs["dec_W_hh"], np.float32),
        np.asarray(inputs["dec_b_ih"], np.float32),
        np.asarray(inputs["dec_b_hh"], np.float32),
        np.asarray(inputs["lin_W"], np.float32),
        np.asarray(inputs["lin_b"], np.float32),
    )
    in_maps = []
    for j in range(NCORES):
        xs = x[LB * j : LB * (j + 1), :t_steps]  # [32, T, 128]
        xT = np.ascontiguousarray(xs.transpose(2, 1, 0)).reshape(128, t_steps * LB)
        m = dict(shared)
        m["xT"] = xT.astype(BF16)
        in_maps.append(m)

    res = run_bass_kernel_spmd(
        nc, in_maps, list(range(NCORES)), trace=trace
    )
    out = np.empty((B, t_steps, I), np.float32)
    for j in range(NCORES):
        o = res.results[j]["out"].astype(np.float32).reshape(128, t_steps, LB)
        out[LB * j : LB * (j + 1)] = o.transpose(2, 1, 0)
    return out, res


def kernel(**inputs):
    out, _ = _run(inputs, T)
    return out


# revision 16
# speedup vs baseline: 1.1976x; 1.0006x over previous
"""Trainium2 Bass kernel: LSTM encoder-decoder (IoT anomaly detector).

Reference semantics (B=256, T=512, I=128, H=256):
  encoder LSTM over x[B,T,I] -> final (h,c); pred_last = sigmoid(h @ lin_W.T + lin_b)
  decoder LSTM run T-1 steps feeding back its own prediction; outputs in
  forward time order [B,T,I].

Sharding: pure data parallelism, batch 256 -> 8 cores x 32.

The recurrence is latency-bound: one dependency cycle (h -> whh matmuls ->
sigmoid -> c update -> tanh -> h) per time step; batch/stream splitting
cannot raise throughput, so the kernel minimizes that cycle:
  - transposed layout: gate/hidden dims on partitions, batch (32) on free.
  - gates split across TWO psum banks: bank A = [f0 f1 i0 i1] (cols 4x32),
    bank B = [g0 g1 o0 o1].  sigmoid(f,i) waits only on bank A's 8 whh
    matmuls; bank B's 8 whh matmuls run while sigmoid(f,i) executes.
  - nonlin chain (all elementwise on DVE, in-order): fc = sig_f*c (runs
    under ACT tanh_g), u = sig_i*G, c = fc+u, ACT tanh_c, h = sig_o*Tc
    (bf16 out, feeds next step's matmuls directly).  S/G/O/Tc/u are bf16
    (2x DVE mode where both operands are 16-bit); c stays fp32 in its own
    PSUM bank (cheaper ACT tanh source, no PE writer so no collisions).
  - bias enters via ONE indicator matmul per bank (lhsT rows 0..3 hi /
    8..11 lo bf16 split), emitted with the x matmuls BEFORE the
    h-dependent whh matmuls so they execute during the previous step's
    nonlinearity.
  - decoder linear: z matmuls emitted right after h; sigmoid (bias via the
    fp32 ACT bias AP) writes the bf16 pred directly into the preds buffer,
    which is also the wih rhs.  preds DMA out as bf16; host converts.
"""

import numpy as np
import ml_dtypes

B, T, I, H = 256, 512, 128, 256
NCORES = 8
LB = B // NCORES  # 32 local batch

BF16 = ml_dtypes.bfloat16

_BUILT = {}


def _build(t_steps):
    import concourse.bass as bass
    import concourse.tile as tile
    from concourse import bacc, mybir

    f32 = mybir.dt.float32
    bf16 = mybir.dt.bfloat16
    AF = mybir.ActivationFunctionType

    nc = bacc.Bacc(
        "TRN2", target_bir_lowering=False, debug=False, num_devices=NCORES
    )

    xT_d = nc.dram_tensor("xT", [128, t_steps * LB], bf16, kind="ExternalInput")
    wih_e_d = nc.dram_tensor("wih_e", [128, 8 * 128], bf16, kind="ExternalInput")
    whh_e_d = nc.dram_tensor("whh_e", [128, 16 * 128], bf16, kind="ExternalInput")
    bfa_e_d = nc.dram_tensor("bfa_e", [128, 128], bf16, kind="ExternalInput")
    bfb_e_d = nc.dram_tensor("bfb_e", [128, 128], bf16, kind="ExternalInput")
    wih_d_d = nc.dram_tensor("wih_d", [128, 8 * 128], bf16, kind="ExternalInput")
    whh_d_d = nc.dram_tensor("whh_d", [128, 16 * 128], bf16, kind="ExternalInput")
    bfa_d_d = nc.dram_tensor("bfa_d", [128, 128], bf16, kind="ExternalInput")
    bfb_d_d = nc.dram_tensor("bfb_d", [128, 128], bf16, kind="ExternalInput")
    wlin_d = nc.dram_tensor("wlin", [128, 2 * 128], bf16, kind="ExternalInput")
    bl_d = nc.dram_tensor("bl", [128, 1], f32, kind="ExternalInput")
    ind_d = nc.dram_tensor("ind", [128, 128], bf16, kind="ExternalInput")
    out_d = nc.dram_tensor("out", [128, t_steps * LB], bf16, kind="ExternalOutput")

    with tile.TileContext(nc) as tc:
        from contextlib import ExitStack

        with ExitStack() as ctx:
            const = ctx.enter_context(tc.tile_pool(name="const", bufs=1))
            work = ctx.enter_context(tc.tile_pool(name="work", bufs=3))
            pa = ctx.enter_context(tc.tile_pool(name="pa", bufs=2, space="PSUM"))
            pb = ctx.enter_context(tc.tile_pool(name="pb", bufs=2, space="PSUM"))
            pl = ctx.enter_context(tc.tile_pool(name="pl", bufs=2, space="PSUM"))
            pc = ctx.enter_context(tc.tile_pool(name="pc", bufs=1, space="PSUM"))
            pd = ctx.enter_context(tc.tile_pool(name="pd", bufs=1, space="PSUM"))

            def load(dram, shape, dt):
                t = const.tile(shape, dt, tag=dram.name)
                nc.sync.dma_start(out=t[:], in_=dram[:])
                return t

            xT = const.tile([128, t_steps * LB], bf16, tag="xT")
            nq = 4 if t_steps % 4 == 0 else 1
            csz = (t_steps // nq) * LB

            # encoder-critical tensors first so step 0 starts ASAP
            bfa_e = load(bfa_e_d, [128, 128], bf16)
            bfb_e = load(bfb_e_d, [128, 128], bf16)
            ind = load(ind_d, [128, 128], bf16)
            wih_e = load(wih_e_d, [128, 8 * 128], bf16)
            whh_e = load(whh_e_d, [128, 16 * 128], bf16)
            for q in range(nq):
                nc.sync.dma_start(
                    out=xT[:, csz * q : csz * (q + 1)],
                    in_=xT_d[:, csz * q : csz * (q + 1)],
                )
            # decoder-only tensors load behind the encoder's first steps
            wih_dd = load(wih_d_d, [128, 8 * 128], bf16)
            whh_dd = load(whh_d_d, [128, 16 * 128], bf16)
            bfa_dd = load(bfa_d_d, [128, 128], bf16)
            bfb_dd = load(bfb_d_d, [128, 128], bf16)
            wlin = load(wlin_d, [128, 2 * 128], bf16)
            bl = load(bl_d, [128, 1], f32)

            preds = const.tile([128, t_steps * LB], bf16, tag="preds")
            c_full = pc.tile([128, 512], f32, tag="c", name="c_full")
            c = c_full[:, 0:64]
            h = const.tile([128, 64], bf16, tag="h")

            dmy = pd.tile([128, 512], f32, tag="dmy")
            fsc = const.tile([128, 1], f32, tag="fsc")

            nc.vector.memset(c[:], 0.0)
            nc.vector.memset(h[:], 0.0)

            cur = {"A": None, "B": None}

            def pslice(t):
                return preds[:, LB * t : LB * (t + 1)]

            def fills(bfa, bfb, x_t=None, wih=None):
                """open gates(t) groups on both banks: bias fill (+ encoder
                x matmuls). No h/pred deps -> run during previous nonlin."""
                ga = pa.tile([128, 512], f32, tag="ga")
                gb = pb.tile([128, 512], f32, tag="gb")
                cur["A"], cur["B"] = ga, gb
                nc.tensor.matmul(ga[:, 0:128], bfa[:], ind[:], start=True, stop=False)
                nc.tensor.matmul(gb[:, 0:128], bfb[:], ind[:], start=True, stop=False)
                if x_t is not None:
                    rhs = xT[:, LB * x_t : LB * (x_t + 1)]
                    for m in range(8):
                        g = cur["A" if m < 4 else "B"]
                        nc.tensor.matmul(
                            g[:, 32 * (m % 4) : 32 * (m % 4) + 32],
                            wih[:, 128 * m : 128 * (m + 1)],
                            rhs,
                            start=False,
                            stop=False,
                        )

            def whh_mms(whh, bank, stop):
                g = cur[bank]
                ms = range(0, 4) if bank == "A" else range(4, 8)
                for k in range(2):
                    for j, m in enumerate(ms):
                        last = stop and k == 1 and j == 3
                        nc.tensor.matmul(
                            g[:, 32 * (m % 4) : 32 * (m % 4) + 32],
                            whh[:, 128 * (8 * k + m) : 128 * (8 * k + m + 1)],
                            h[:, 32 * k : 32 * k + 32],
                            start=False,
                            stop=last,
                        )

            def wih_mms(wih, in_slot, bank):
                g = cur[bank]
                rhs = pslice(in_slot)
                ms = range(0, 4) if bank == "A" else range(4, 8)
                for j, m in enumerate(ms):
                    nc.tensor.matmul(
                        g[:, 32 * (m % 4) : 32 * (m % 4) + 32],
                        wih[:, 128 * m : 128 * (m + 1)],
                        rhs,
                        start=False,
                        stop=(j == 3),
                    )

            def filler():
                # dummy 1-col matmul on h (runs on PE before the whh burst)
                # + 1-col ACT op: ScalarE is mid-op when bank A closes, so
                # sigma(f,i)'s dispatch overlaps the filler tail (~90ns).
                nc.tensor.matmul(
                    dmy[:, 0:1], ind[:], h[:, 0:1], start=True, stop=True
                )
                nc.scalar.activation(fsc[:], dmy[:, 0:1], AF.Sigmoid)

            def nonlin():
                ga, gb = cur["A"], cur["B"]
                S = work.tile([128, 128], bf16, tag="S")
                nc.scalar.activation(S[:], ga[:, 0:128], AF.Sigmoid)
                G = work.tile([128, 64], bf16, tag="G")
                nc.scalar.activation(G[:], gb[:, 0:64], AF.Tanh)
                O = work.tile([128, 64], bf16, tag="O")
                nc.scalar.activation(O[:], gb[:, 64:128], AF.Sigmoid)
                fc = work.tile([128, 64], bf16, tag="fc")
                nc.vector.tensor_mul(fc[:], S[:, 0:64], c[:])
                u = work.tile([128, 64], bf16, tag="u")
                nc.vector.tensor_mul(u[:], S[:, 64:128], G[:])
                nc.vector.tensor_add(c[:], fc[:], u[:])
                Tc = work.tile([128, 64], bf16, tag="Tc")
                nc.scalar.activation(Tc[:], c[:], AF.Tanh)
                nc.vector.tensor_mul(h[:], O[:], Tc[:])

            def lin_block(out_slot):
                lz = pl.tile([128, 512], f32, tag="lz")
                nc.tensor.matmul(
                    lz[:, 0:LB], wlin[:, 0:128], h[:, 0:32],
                    start=True, stop=False,
                )
                nc.tensor.matmul(
                    lz[:, 0:LB], wlin[:, 128:256], h[:, 32:64],
                    start=False, stop=True,
                )
                nc.scalar.activation(
                    pslice(out_slot), lz[:, 0:LB], AF.Sigmoid, bias=bl[:]
                )

            # ---- encoder ----
            fills(bfa_e, bfb_e, x_t=0, wih=wih_e)
            for t in range(t_steps):
                whh_mms(whh_e, "A", stop=True)
                whh_mms(whh_e, "B", stop=True)
                nonlin()
                if t + 1 < t_steps:
                    fills(bfa_e, bfb_e, x_t=t + 1, wih=wih_e)

            # pred at last slot from encoder final h; open first decoder banks
            fills(bfa_dd, bfb_dd)
            lin_block(t_steps - 1)

            # ---- decoder: iteration k consumes pred slot T-1-k, writes T-2-k
            for k in range(t_steps - 1):
                whh_mms(whh_dd, "A", stop=False)
                whh_mms(whh_dd, "B", stop=False)
                wih_mms(wih_dd, t_steps - 1 - k, "A")
                wih_mms(wih_dd, t_steps - 1 - k, "B")
                nonlin()
                if k + 1 < t_steps - 1:
                    fills(bfa_dd, bfb_dd)
                lin_block(t_steps - 2 - k)
                if t_steps >= 8 and k == t_steps - 2 - (t_steps // 2):
                    half = (t_steps // 2) * LB
                    nc.sync.dma_start(
                        out=out_d[:, half:], in_=preds[:, half:]
                    )

            half = (t_steps // 2) * LB if t_steps >= 8 else 0
            nc.sync.dma_start(out=out_d[:, 0:half] if half else out_d[:],
                              in_=preds[:, 0:half] if half else preds[:])

    nc.compile()
    return nc


def _get(t_steps):
    if t_steps not in _BUILT:
        _BUILT[t_steps] = _build(t_steps)
    return _BUILT[t_steps]


def _pack_weights(enc_W_ih, enc_W_hh, enc_b_ih, enc_b_hh,
                  dec_W_ih, dec_W_hh, dec_b_ih, dec_b_hh, lin_W, lin_b):
    # chunk order [f0 f1 i0 i1 g0 g1 o0 o1]; torch gate rows are [i f g o].
    perm = np.r_[H : 2 * H, 0:H, 2 * H : 3 * H, 3 * H : 4 * H]

    def pack_ih(W):  # [4H, I] -> [128, 8*128] lhsT tiles
        Wp = W[perm].reshape(8, 128, I)
        return np.concatenate([Wp[m].T for m in range(8)], axis=1).astype(BF16)

    def pack_hh(W):  # [4H, H] -> [128, 16*128], tile (k,m) at col 128*(8k+m)
        Wp = W[perm]
        tiles = [
            Wp[128 * m : 128 * (m + 1), 128 * k : 128 * (k + 1)].T
            for k in range(2)
            for m in range(8)
        ]
        return np.concatenate(tiles, axis=1).astype(BF16)

    def pack_bias_fill(b, bank):  # [4H] -> [128,128]: rows 0..3 hi, 8..11 lo
        bp = b[perm].astype(np.float32)
        out = np.zeros((128, 128), np.float32)
        base = 0 if bank == "A" else 4
        for j in range(4):
            chunk = bp[128 * (base + j) : 128 * (base + j + 1)]
            hi = chunk.astype(BF16).astype(np.float32)
            out[j, :] = hi
            out[8 + j, :] = chunk - hi
        return out.astype(BF16)

    ind = np.zeros((128, 128), np.float32)
    for j in range(4):
        ind[j, 32 * j : 32 * j + 32] = 1.0
        ind[8 + j, 32 * j : 32 * j + 32] = 1.0

    wlin = np.concatenate(
        [lin_W[:, 0:128].T, lin_W[:, 128:256].T], axis=1
    ).astype(BF16)

    b_e = enc_b_ih + enc_b_hh
    b_d = dec_b_ih + dec_b_hh
    return {
        "wih_e": pack_ih(enc_W_ih),
        "whh_e": pack_hh(enc_W_hh),
        "bfa_e": pack_bias_fill(b_e, "A"),
        "bfb_e": pack_bias_fill(b_e, "B"),
        "wih_d": pack_ih(dec_W_ih),
        "whh_d": pack_hh(dec_W_hh),
        "bfa_d": pack_bias_fill(b_d, "A"),
        "bfb_d": pack_bias_fill(b_d, "B"),
        "wlin": wlin,
        "bl": lin_b.astype(np.float32).reshape(128, 1),
        "ind": ind.astype(BF16),
    }


def _run(inputs, t_steps, trace=False):
    from concourse.bass_utils import run_bass_kernel_spmd

    nc = _get(t_steps)
    x = np.asarray(inputs["x"], np.float32)
    shared = _pack_weights(
        np.asarray(inputs["enc_W_ih"], np.float32),
        np.asarray(inputs["enc_W_hh"], np.float32),
        np.asarray(inputs["enc_b_ih"], np.float32),
        np.asarray(inputs["enc_b_hh"], np.float32),
        np.asarray(inputs["dec_W_ih"], np.float32),
        np.asarray(inputs["dec_W_hh"], np.float32),
        np.asarray(inputs["dec_b_ih"], np.float32),
        np.asarray(inputs["dec_b_hh"], np.float32),
        np.asarray(inputs["lin_W"], np.float32),
        np.asarray(inputs["lin_b"], np.float32),
    )
    in_maps = []
    for j in range(NCORES):
        xs = x[LB * j : LB * (j + 1), :t_steps]  # [32, T, 128]
        xT = np.ascontiguousarray(xs.transpose(2, 1, 0)).reshape(128, t_steps * LB)
        m = dict(shared)
        m["xT"] = xT.astype(BF16)
        in_maps.append(m)

    res = run_bass_kernel_spmd(
        nc, in_maps, list(range(NCORES)), trace=trace
    )
    out = np.empty((B, t_steps, I), np.float32)
    for j in range(NCORES):
        o = res.results[j]["out"].astype(np.float32).reshape(128, t_steps, LB)
        out[LB * j : LB * (j + 1)] = o.transpose(2, 1, 0)
    return out, res


def kernel(**inputs):
    out, _ = _run(inputs, T)
    return out
